# revision 5
# baseline (speedup 1.0000x reference)
"""Trainium2 Bass kernel: decode-step attention with static KV cache (GQA).

Problem shapes (hardcoded):
  x        [16, 1, 4096]      activations (B=16, QLEN=1, DIM=4096)
  cache_k  [16, 8192, 8, 128] K cache (PREFIX=8192, HKV=8, HD=128)
  cache_v  [16, 8192, 8, 128]
  wq       [4096, 4096]  (H*HD, DIM), H=32
  wk/wv    [1024, 4096]
  wo       [4096, 4096]  (DIM, H*HD)
  out      [16, 1, 4096]

Sharding: tensor-parallel over the kv-head axis. Core c owns kv head c and
q heads 4c..4c+3; weights are column/row-sliced per core, the KV slice is
extracted per core on the host (K transposed to [d, t] with an interleaved
column order, see below). Each core computes a partial output in transposed
layout [4096, 16]; the host transposes and sums the 8 partials.

The kernel is DMA-bound: per core it must read 155.4 MB (128 MB f32 KV
cache + 21 MB weight slices) against a ~430 GB/s per-core ceiling
(16 SDMA engines x 27 GiB/s).  Everything else is scheduled to hide
behind that stream:
  - All loads are SWDGE cast DMAs (f32 HBM -> f16 SBUF) issued in
    consumption order: x, wk, wv, wq (one 8.4MB-read load), then K/V per
    batch (3 batches deep), and last the 4 column-chunks of wo.
  - The output projection runs in transposed form, outT[n,b] =
    sum_c wo_c[d,n]^T @ AT_c[d,b], as 128 weight-load-bound matmuls
    gated per wo column-chunk, so only ~2us of work trails the final
    DMA byte.

PE dtype strategy: fp32 matmuls on TRN2 run as two half-speed passes and
fp32 weight loads get no FWL, so K, V, q and P are cast to float16
(all values are O(10), P=exp(score)<~1100, so fp16 is exact to ~5e-4
overall) while every accumulation stays fp32 in PSUM.

t-ordering: V loads contiguously as [128, (n d)] with t = 64*p + n
(p = partition, n = tile index).  The host permutes K's columns to the
same order, so score tiles and V tiles agree on partition<->t mapping.
The softmax denominator comes from a ones-column matmul over P (plus a
tiny [1,4]->[4,1] PE transpose for the per-head reciprocal).

Per-core dataflow:
  phase 0: q/k_new/v_new projections (f16 PE), transposes to get
           qT[d,(h,b)], kT_new[d,b], v_new rows; cast to f16.
  phase 1 (per b): 64+1 score matmuls (f16) -> PSUM f32 [t-tile, h];
           exp (ACT, scale=1/sqrt(128)) -> P f16; 64+1 PV matmuls
           accumulate [h, d] in PSUM f32; ones-matmul gives
           denominators; scale by reciprocal; transpose to AT[d,(h,b)].
  phase 2: outT[128n, 16b] tiles accumulated over the 4 head chunks,
           copied to SBUF, stored per wo-chunk via HWDGE.
"""

import sys

_REPO = "/opt/trn_rl_repo"
if _REPO not in sys.path:
    sys.path.insert(0, _REPO)

import numpy as np

import concourse.bacc as bacc
import concourse.mybir as mybir
import concourse.tile as tile
from concourse.bass_utils import run_bass_kernel_spmd
from concourse.masks import make_identity

B = 16          # batch
T = 8192        # prefix length in cache
NT = T // 128   # 64 K/V tiles per batch
HD = 128        # head dim
HQ = 4          # q heads per core
DIM = 4096
NDT = DIM // 128  # 32 contraction tiles for the projections
NCORES = 8
F32 = mybir.dt.float32
F16 = mybir.dt.float16
SCALE = 1.0 / float(np.sqrt(128.0))
SW = 4 * NT + 4   # score tile width: 64 cache tiles + new token, 4 heads each
KV_DEPTH = 3      # K/V batches in flight

Exp = mybir.ActivationFunctionType.Exp
Mult = mybir.AluOpType.mult


def _build_nc():
    nc = bacc.Bacc("TRN2", target_bir_lowering=False, debug=False)

    xT = nc.dram_tensor("xT", [DIM, B], F32, kind="ExternalInput")
    wqT = nc.dram_tensor("wqT", [DIM, HQ * HD], F32, kind="ExternalInput")
    wkT = nc.dram_tensor("wkT", [DIM, HD], F32, kind="ExternalInput")
    wvT = nc.dram_tensor("wvT", [DIM, HD], F32, kind="ExternalInput")
    woT = nc.dram_tensor("woT", [HQ * HD, DIM], F32, kind="ExternalInput")
    kT = nc.dram_tensor("kT", [B, HD, T], F32, kind="ExternalInput")
    v = nc.dram_tensor("v", [B, T, HD], F32, kind="ExternalInput")
    # out holds outT tiles in SBUF layout: out[p, nt*B + b] = y[nt*128+p, b]
    out = nc.dram_tensor("out", [128, NDT * B], F32, kind="ExternalOutput")

    with tile.TileContext(nc) as tc:
        _emit(nc, tc, xT, wqT, wkT, wvT, woT, kT, v, out)
    nc.compile()
    return nc


def _emit(nc, tc, xT, wqT, wkT, wvT, woT, kT, v, out):
    from contextlib import ExitStack

    with ExitStack() as ctx:
        const = ctx.enter_context(tc.tile_pool(name="const", bufs=1))
        kpool = ctx.enter_context(tc.tile_pool(name="kpool", bufs=KV_DEPTH))
        vpool = ctx.enter_context(tc.tile_pool(name="vpool", bufs=KV_DEPTH))

        ident = const.tile([16, 16], F32, tag="ident")

        # SWDGE queue order == emission order: x, wk, wv, wq, K/V, ..., wo.
        # x^T in f16: [128, (dt, b)] (SWDGE cast load)
        xs_h = const.tile([128, NDT * B], F16, tag="xs_h")
        nc.gpsimd.dma_start(
            xs_h[:].rearrange("p (t b) -> p t b", b=B),
            xT[:].rearrange("(t p) b -> p t b", p=128),
        )
        # wk/wv resident in f16 (one 2MB-read SWDGE cast DMA each)
        wk_h = const.tile([128, NDT * HD], F16, tag="wk_h")
        nc.gpsimd.dma_start(
            wk_h[:].rearrange("p (t n) -> p t n", n=HD),
            wkT[:].rearrange("(t p) n -> p t n", p=128),
        )
        wv_h = const.tile([128, NDT * HD], F16, tag="wv_h")
        nc.gpsimd.dma_start(
            wv_h[:].rearrange("p (t n) -> p t n", n=HD),
            wvT[:].rearrange("(t p) n -> p t n", p=128),
        )
        # wq resident in f16: one 8.4MB-read cast DMA
        wq_h = const.tile([128, NDT * HQ * HD], F16, tag="wq_h")
        nc.gpsimd.dma_start(
            wq_h[:].rearrange("p (t n) -> p t n", n=HQ * HD),
            wqT[:].rearrange("(t p) n -> p t n", p=128),
        )

        # K/V cast loads, KV_DEPTH batches deep; pre-issue the first window
        # so the cache stream queues directly behind the weight loads.
        ktiles, vtiles = {}, {}

        def issue_kv(b):
            ktb = kpool.tile([128, T], F16, tag="ktb")
            nc.gpsimd.dma_start(ktb[:], kT[b])
            vb = vpool.tile([128, T], F16, tag="vb")
            nc.gpsimd.dma_start(
                vb[:], v[b].rearrange("(p n) d -> p (n d)", p=128)
            )
            ktiles[b], vtiles[b] = ktb, vb

        for b in range(KV_DEPTH):
            issue_kv(b)

        QT = const.tile([128, HQ * B], F32, tag="QT")       # [d, (h,b)] fp32
        QTh = const.tile([128, HQ * B], F16, tag="QTh")     # fp16 copy
        KTnh = const.tile([128, B], F16, tag="KTnh")        # new-token K^T f16
        vrowh = const.tile([1, B * HD], F16, tag="vrowh")   # new-token V rows f16
        AT = const.tile([128, HQ * B], F16, tag="AT")       # attn out^T f16
        wo_h = const.tile([128, 4 * DIM], F16, tag="wo_h")  # resident f16 wo
        q_s = const.tile([B, HQ * HD], F32, tag="q_s")
        kn_s = const.tile([B, HD], F32, tag="kn_s")
        vn_s = const.tile([B, HD], F32, tag="vn_s")
        ones_h = const.tile([128, 1], F16, tag="ones_h")
        outT_s = const.tile([128, NDT * B], F32, tag="outT_s")

        make_identity(nc, ident[:])
        nc.vector.memset(ones_h[:], 1.0)

        # ---------------- phase 0: projections (f16 PE) ----------------
        with tc.tile_pool(name="psum0", bufs=1, space="PSUM") as pp0:
            qp = pp0.tile([B, HQ * HD], F32, tag="qp")
            knp = pp0.tile([B, HD], F32, tag="knp")
            vnp = pp0.tile([B, HD], F32, tag="vnp")

            for dt in range(NDT):
                nc.tensor.matmul(
                    qp[:], xs_h[:, dt * B:(dt + 1) * B],
                    wq_h[:, dt * HQ * HD:(dt + 1) * HQ * HD],
                    start=(dt == 0), stop=(dt == NDT - 1),
                )
            for dt in range(NDT):
                nc.tensor.matmul(
                    knp[:], xs_h[:, dt * B:(dt + 1) * B],
                    wk_h[:, dt * HD:(dt + 1) * HD],
                    start=(dt == 0), stop=(dt == NDT - 1),
                )
            for dt in range(NDT):
                nc.tensor.matmul(
                    vnp[:], xs_h[:, dt * B:(dt + 1) * B],
                    wv_h[:, dt * HD:(dt + 1) * HD],
                    start=(dt == 0), stop=(dt == NDT - 1),
                )

            nc.vector.tensor_copy(q_s[:], qp[:])
            nc.vector.tensor_copy(kn_s[:], knp[:])
            nc.vector.tensor_copy(vn_s[:], vnp[:])

            # v_new rows (f16) flattened onto partition 0 (SWDGE cast DMA)
            nc.gpsimd.dma_start(
                vrowh[:].rearrange("p (b c) -> p b c", c=HD)[0:1, :, :],
                vn_s[:],
            )

            # transposes: q [16,512] -> QT [128, (h,b)]; k_new -> KTn (f16)
            for h in range(HQ):
                tp = pp0.tile([128, B], F32, tag="tp", bufs=2)
                nc.tensor.transpose(
                    tp[:], q_s[:, h * HD:(h + 1) * HD], ident[:]
                )
                nc.vector.tensor_copy(QT[:, h * B:(h + 1) * B], tp[:])
            tpk = pp0.tile([128, B], F32, tag="tp", bufs=2)
            nc.tensor.transpose(tpk[:], kn_s[:], ident[:])
            nc.vector.tensor_copy(KTnh[:], tpk[:])
            nc.vector.tensor_copy(QTh[:], QT[:])

        # ---------------- phase 1: attention over the cache ----------------
        QTh3 = QTh[:].rearrange("p (h b) -> p b h", b=B)   # [128, b, 4]
        vrowh3 = vrowh[:].rearrange("p (b c) -> p b c", c=HD)
        AT3 = AT[:].rearrange("p (h b) -> p b h", b=B)

        with (
            tc.tile_pool(name="ptpool", bufs=2) as ptpool,
            tc.tile_pool(name="small", bufs=2) as small,
            tc.tile_pool(name="stpsum", bufs=2, space="PSUM") as stpsum,
            tc.tile_pool(name="opsum", bufs=2, space="PSUM") as opsum,
            tc.tile_pool(name="denpsum", bufs=1, space="PSUM") as denpsum,
            tc.tile_pool(name="ttpsum", bufs=2, space="PSUM") as ttpsum,
        ):
            for b in range(B):
                ktb = ktiles.pop(b)
                vb = vtiles.pop(b)

                # scores^T tiles: [t'(128), h(4)] per cache tile + new token
                stp = stpsum.tile([128, SW], F32, tag="stp")
                qb = QTh3[:, b, :]
                nc.tensor.matmul(
                    stp[0:1, 4 * NT:SW], KTnh[:, b:b + 1], qb,
                    start=True, stop=True,
                )
                for n in range(NT):
                    nc.tensor.matmul(
                        stp[:, 4 * n:4 * n + 4],
                        ktb[:, 128 * n:128 * (n + 1)],
                        qb,
                        start=True, stop=True,
                    )

                pt = ptpool.tile([128, SW], F16, tag="pt")
                nc.scalar.activation(pt[:, 0:4 * NT], stp[:, 0:4 * NT], Exp, scale=SCALE)
                nc.scalar.activation(
                    pt[0:1, 4 * NT:SW], stp[0:1, 4 * NT:SW], Exp, scale=SCALE,
                )

                # out^T [h(4), 128]: accumulate cache tiles + new token
                op = opsum.tile([HQ, HD], F32, tag="op")
                nc.tensor.matmul(
                    op[:], pt[0:1, 4 * NT:SW], vrowh3[0:1, b, :],
                    start=True, stop=False,
                )
                for n in range(NT):
                    nc.tensor.matmul(
                        op[:],
                        pt[:, 4 * n:4 * n + 4],
                        vb[:, 128 * n:128 * (n + 1)],
                        start=False, stop=(n == NT - 1),
                    )

                # softmax denominators: ones.T @ P -> [1, (g h)], reduce g
                dps = denpsum.tile([1, SW], F32, tag="dps")
                nc.tensor.matmul(
                    dps[0:1, 0:4 * NT], ones_h[:], pt[:, 0:4 * NT],
                    start=True, stop=True,
                )
                nc.tensor.matmul(
                    dps[0:1, 4 * NT:SW], ones_h[0:1, 0:1], pt[0:1, 4 * NT:SW],
                    start=True, stop=True,
                )
                dred = small.tile([1, HQ], F32, tag="dred")
                nc.vector.reduce_sum(
                    dred[:].rearrange("p h -> p h ()"),
                    dps[:].rearrange("p (g h) -> p h g", h=HQ),
                    axis=mybir.AxisListType.X,
                )
                dent = ttpsum.tile([HQ, 1], F32, tag="tt")
                nc.tensor.matmul(dent[:], dred[:], ident[0:1, 0:1],
                                 start=True, stop=True)

                rc = small.tile([HQ, 1], F32, tag="rc")
                nc.vector.reciprocal(rc[:], dent[:])
                ao = small.tile([HQ, HD], F32, tag="ao")
                nc.vector.tensor_scalar(
                    out=ao[:], in0=op[:], scalar1=rc[:], scalar2=None, op0=Mult
                )

                tt = ttpsum.tile([128, HQ], F32, tag="tt")
                nc.tensor.transpose(tt[:], ao[:], ident[0:HQ, 0:HQ])
                nc.vector.tensor_copy(AT3[:, b, :], tt[:])

                if b + KV_DEPTH < B:
                    issue_kv(b + KV_DEPTH)

        # wo f16 chunks queue behind the whole K/V FIFO, contiguous reads
        # (8KB per partition), n-low halves first across all 4 head chunks
        # so phase-2 work for the first output columns starts while the
        # n-high halves are still streaming.
        wo_src = woT[:].rearrange("(c p) n -> c p n", p=128)
        for half in range(2):
            for c in range(4):
                nc.gpsimd.dma_start(
                    wo_h[:, c * DIM + 2048 * half:c * DIM + 2048 * (half + 1)],
                    wo_src[c, :, 2048 * half:2048 * (half + 1)],
                )

        # ---------------- phase 2: output projection, transposed -----------
        # outT[128n, 16b] per n-tile = sum_c wo_h[:, c*DIM+n-range]^T @ AT_c.
        with tc.tile_pool(name="otpsum", bufs=4, space="PSUM") as otpsum:
            for half in range(2):
                for t in range(16):
                    nt = 16 * half + t
                    otp = otpsum.tile([128, B], F32, tag="otp")
                    for c in range(HQ):
                        nc.tensor.matmul(
                            otp[:],
                            wo_h[:, c * DIM + 128 * nt:c * DIM + 128 * (nt + 1)],
                            AT[:, B * c:B * (c + 1)],
                            start=(c == 0), stop=(c == HQ - 1),
                        )
                    nc.vector.tensor_copy(outT_s[:, B * nt:B * (nt + 1)], otp[:])
                nc.sync.dma_start(
                    out[:, B * 16 * half:B * 16 * (half + 1)],
                    outT_s[:, B * 16 * half:B * 16 * (half + 1)],
                )

_NC = None


def _get_nc():
    global _NC
    if _NC is None:
        _NC = _build_nc()
    return _NC


def make_in_maps(inputs):
    x = np.ascontiguousarray(np.asarray(inputs["x"], dtype=np.float32))
    ck = np.asarray(inputs["cache_k"], dtype=np.float32)
    cv = np.asarray(inputs["cache_v"], dtype=np.float32)
    wq = np.asarray(inputs["wq"], dtype=np.float32)
    wk = np.asarray(inputs["wk"], dtype=np.float32)
    wv = np.asarray(inputs["wv"], dtype=np.float32)
    wo = np.asarray(inputs["wo"], dtype=np.float32)

    xT = np.ascontiguousarray(x.reshape(B, DIM).T)
    wqT = np.ascontiguousarray(wq.T)    # [DIM, H*HD]
    wkT = np.ascontiguousarray(wk.T)    # [DIM, HKV*HD]
    wvT = np.ascontiguousarray(wv.T)

    in_maps = []
    for c in range(NCORES):
        hq0 = HQ * HD * c
        # K^T with columns permuted to the t = 64*p + n interleaved order
        # (matches V's natural contiguous-load partition mapping).
        kTc = ck[:, :, c, :].transpose(0, 2, 1)           # [B, 128d, 8192t]
        kTc = np.ascontiguousarray(
            kTc.reshape(B, HD, 128, NT).transpose(0, 1, 3, 2).reshape(B, HD, T)
        )
        in_maps.append({
            "xT": xT,
            "wqT": np.ascontiguousarray(wqT[:, hq0:hq0 + HQ * HD]),
            "wkT": np.ascontiguousarray(wkT[:, HD * c:HD * (c + 1)]),
            "wvT": np.ascontiguousarray(wvT[:, HD * c:HD * (c + 1)]),
            "woT": np.ascontiguousarray(wo[:, hq0:hq0 + HQ * HD].T),
            "kT": kTc,
            "v": np.ascontiguousarray(cv[:, :, c, :]),
        })
    return in_maps


def run(in_maps, trace=False):
    nc = _get_nc()
    return run_bass_kernel_spmd(nc, in_maps, list(range(NCORES)), trace=trace)


def unscramble_out(o):
    """[128, NDT*B] outT tiles -> [B, DIM] partial output."""
    return np.asarray(o).reshape(128, NDT, B).transpose(1, 0, 2).reshape(DIM, B).T


def kernel(**inputs):
    res = run(make_in_maps(inputs)).results
    acc = np.zeros((B, DIM), dtype=np.float64)
    for r in res:
        acc += unscramble_out(r["out"])
    return acc.astype(np.float32).reshape(B, 1, DIM)


# revision 7
# speedup vs baseline: 1.1105x; 1.1105x over previous
"""Trainium2 Bass kernel: decode-step attention with static KV cache (GQA).

Problem shapes (hardcoded):
  x        [16, 1, 4096]      activations (B=16, QLEN=1, DIM=4096)
  cache_k  [16, 8192, 8, 128] K cache (PREFIX=8192, HKV=8, HD=128)
  cache_v  [16, 8192, 8, 128]
  wq       [4096, 4096]  (H*HD, DIM), H=32
  wk/wv    [1024, 4096]
  wo       [4096, 4096]  (DIM, H*HD)
  out      [16, 1, 4096]

Sharding: tensor-parallel over the kv-head axis. Core c owns kv head c and
q heads 4c..4c+3; weights are column/row-sliced per core, the KV slice is
extracted per core on the host (K transposed to [d, t] with an interleaved
column order, see below). Each core computes a partial output in transposed
layout [4096, 16]; the host transposes and sums the 8 partials.

The kernel is DMA-bound: per core it must read 155.4 MB (128 MB f32 KV
cache + 21 MB weight slices) against a ~430 GB/s per-core ceiling
(16 SDMA engines x 27 GiB/s).  Everything else is scheduled to hide
behind that stream:
  - All loads are SWDGE cast DMAs (f32 HBM -> f16 SBUF) issued in
    consumption order: x, wk, wv, wq (one 8.4MB-read load), then K/V per
    batch (3 batches deep), and last the 4 column-chunks of wo.
  - The output projection runs in transposed form, outT[n,b] =
    sum_c wo_c[d,n]^T @ AT_c[d,b], as 128 weight-load-bound matmuls
    gated per wo column-chunk, so only ~2us of work trails the final
    DMA byte.

PE dtype strategy: fp32 matmuls on TRN2 run as two half-speed passes and
fp32 weight loads get no FWL, so K, V, q and P are cast to float16
(all values are O(10), P=exp(score)<~1100, so fp16 is exact to ~5e-4
overall) while every accumulation stays fp32 in PSUM.

t-ordering: V loads contiguously as [128, (n d)] with t = 64*p + n
(p = partition, n = tile index).  The host permutes K's columns to the
same order, so score tiles and V tiles agree on partition<->t mapping.
The softmax denominator comes from a ones-column matmul over P (plus a
tiny [1,4]->[4,1] PE transpose for the per-head reciprocal).

Per-core dataflow:
  phase 0: q/k_new/v_new projections (f16 PE), transposes to get
           qT[d,(h,b)], kT_new[d,b], v_new rows; cast to f16.
  phase 1 (per b): 64+1 score matmuls (f16) -> PSUM f32 [t-tile, h];
           exp (ACT, scale=1/sqrt(128)) -> P f16; 64+1 PV matmuls
           accumulate [h, d] in PSUM f32; ones-matmul gives
           denominators; scale by reciprocal; transpose to AT[d,(h,b)].
  phase 2: outT[128n, 16b] tiles accumulated over the 4 head chunks,
           copied to SBUF, stored per wo-chunk via HWDGE.
"""

import sys

_REPO = "/opt/trn_rl_repo"
if _REPO not in sys.path:
    sys.path.insert(0, _REPO)

import numpy as np

import concourse.bacc as bacc
import concourse.mybir as mybir
import concourse.tile as tile
from concourse.bass_utils import run_bass_kernel_spmd
from concourse.masks import make_identity

B = 16          # batch
T = 8192        # prefix length in cache
NT = T // 128   # 64 K/V tiles per batch
HD = 128        # head dim
HQ = 4          # q heads per core
DIM = 4096
NDT = DIM // 128  # 32 contraction tiles for the projections
NCORES = 8
F32 = mybir.dt.float32
F16 = mybir.dt.float16
SCALE = 1.0 / float(np.sqrt(128.0))
SW = 4 * NT + 4   # score tile width: 64 cache tiles + new token, 4 heads each
KV_DEPTH = 2      # K/V batches in flight

Exp = mybir.ActivationFunctionType.Exp
Mult = mybir.AluOpType.mult


def _build_nc():
    nc = bacc.Bacc("TRN2", target_bir_lowering=False, debug=False)

    xT = nc.dram_tensor("xT", [DIM, B], F32, kind="ExternalInput")
    wqT = nc.dram_tensor("wqT", [DIM, HQ * HD], F32, kind="ExternalInput")
    wkT = nc.dram_tensor("wkT", [DIM, HD], F32, kind="ExternalInput")
    wvT = nc.dram_tensor("wvT", [DIM, HD], F32, kind="ExternalInput")
    woT = nc.dram_tensor("woT", [HQ * HD, DIM], F32, kind="ExternalInput")
    kT = nc.dram_tensor("kT", [B, HD, T], F32, kind="ExternalInput")
    v = nc.dram_tensor("v", [B, T, HD], F32, kind="ExternalInput")
    # out holds outT tiles in SBUF layout: out[p, nt*B + b] = y[nt*128+p, b]
    out = nc.dram_tensor("out", [128, NDT * B], F32, kind="ExternalOutput")

    with tile.TileContext(nc) as tc:
        _emit(nc, tc, xT, wqT, wkT, wvT, woT, kT, v, out)
    nc.compile()
    return nc


def _emit(nc, tc, xT, wqT, wkT, wvT, woT, kT, v, out):
    from contextlib import ExitStack

    with ExitStack() as ctx:
        const = ctx.enter_context(tc.tile_pool(name="const", bufs=1))
        kpool = ctx.enter_context(tc.tile_pool(name="kpool", bufs=KV_DEPTH))
        vpool = ctx.enter_context(tc.tile_pool(name="vpool", bufs=KV_DEPTH))

        ident = const.tile([16, 16], F32, tag="ident")

        # wk/wv via HWDGE f32 (first byte ~0.6us, fills the ~10us SWDGE/Q7
        # boot window), cast to f16 on the otherwise-idle vector engine.
        wk_f = const.tile([128, NDT * HD], F32, tag="wk_f")
        nc.sync.dma_start(
            wk_f[:].rearrange("p (t n) -> p t n", n=HD),
            wkT[:].rearrange("(t p) n -> p t n", p=128),
        )
        wv_f = const.tile([128, NDT * HD], F32, tag="wv_f")
        nc.sync.dma_start(
            wv_f[:].rearrange("p (t n) -> p t n", n=HD),
            wvT[:].rearrange("(t p) n -> p t n", p=128),
        )

        # SWDGE queue order == emission order: x, wq, K/V, ..., wo.
        # x^T in f16: [128, (dt, b)] (SWDGE cast load)
        xs_h = const.tile([128, NDT * B], F16, tag="xs_h")
        nc.gpsimd.dma_start(
            xs_h[:].rearrange("p (t b) -> p t b", b=B),
            xT[:].rearrange("(t p) b -> p t b", p=128),
        )
        # wq resident in f16: one 8.4MB-read cast DMA
        wq_h = const.tile([128, NDT * HQ * HD], F16, tag="wq_h")
        nc.gpsimd.dma_start(
            wq_h[:].rearrange("p (t n) -> p t n", n=HQ * HD),
            wqT[:].rearrange("(t p) n -> p t n", p=128),
        )
        wk_h = const.tile([128, NDT * HD], F16, tag="wk_h")
        nc.vector.tensor_copy(wk_h[:], wk_f[:])
        wv_h = const.tile([128, NDT * HD], F16, tag="wv_h")
        nc.vector.tensor_copy(wv_h[:], wv_f[:])

        # K/V cast loads, KV_DEPTH batches deep; pre-issue the first window
        # so the cache stream queues directly behind the weight loads.
        ktiles, vtiles = {}, {}

        def issue_kv(b):
            ktb = kpool.tile([128, T], F16, tag="ktb")
            nc.gpsimd.dma_start(ktb[:], kT[b])
            vb = vpool.tile([128, T], F16, tag="vb")
            nc.gpsimd.dma_start(
                vb[:], v[b].rearrange("(p n) d -> p (n d)", p=128)
            )
            ktiles[b], vtiles[b] = ktb, vb

        for b in range(KV_DEPTH):
            issue_kv(b)

        QT = const.tile([128, HQ * B], F32, tag="QT")       # [d, (h,b)] fp32
        QTh = const.tile([128, HQ * B], F16, tag="QTh")     # fp16 copy
        KTnh = const.tile([128, B], F16, tag="KTnh")        # new-token K^T f16
        vrowh = const.tile([1, B * HD], F16, tag="vrowh")   # new-token V rows f16
        AT = const.tile([128, HQ * B], F16, tag="AT")       # attn out^T f16
        wo_h = const.tile([128, 4 * DIM], F16, tag="wo_h")  # resident f16 wo
        q_s = const.tile([B, HQ * HD], F32, tag="q_s")
        kn_s = const.tile([B, HD], F32, tag="kn_s")
        vn_s = const.tile([B, HD], F32, tag="vn_s")
        ones_h = const.tile([128, 1], F16, tag="ones_h")
        outT_s = const.tile([128, NDT * B], F32, tag="outT_s")

        make_identity(nc, ident[:])
        nc.vector.memset(ones_h[:], 1.0)

        # ---------------- phase 0: projections (f16 PE) ----------------
        with tc.tile_pool(name="psum0", bufs=1, space="PSUM") as pp0:
            qp = pp0.tile([B, HQ * HD], F32, tag="qp")
            knp = pp0.tile([B, HD], F32, tag="knp")
            vnp = pp0.tile([B, HD], F32, tag="vnp")

            for dt in range(NDT):
                nc.tensor.matmul(
                    qp[:], xs_h[:, dt * B:(dt + 1) * B],
                    wq_h[:, dt * HQ * HD:(dt + 1) * HQ * HD],
                    start=(dt == 0), stop=(dt == NDT - 1),
                )
            for dt in range(NDT):
                nc.tensor.matmul(
                    knp[:], xs_h[:, dt * B:(dt + 1) * B],
                    wk_h[:, dt * HD:(dt + 1) * HD],
                    start=(dt == 0), stop=(dt == NDT - 1),
                )
            for dt in range(NDT):
                nc.tensor.matmul(
                    vnp[:], xs_h[:, dt * B:(dt + 1) * B],
                    wv_h[:, dt * HD:(dt + 1) * HD],
                    start=(dt == 0), stop=(dt == NDT - 1),
                )

            nc.vector.tensor_copy(q_s[:], qp[:])
            nc.vector.tensor_copy(kn_s[:], knp[:])
            nc.vector.tensor_copy(vn_s[:], vnp[:])

            # v_new rows (f16) flattened onto partition 0 (SWDGE cast DMA)
            nc.gpsimd.dma_start(
                vrowh[:].rearrange("p (b c) -> p b c", c=HD)[0:1, :, :],
                vn_s[:],
            )

            # transposes: q [16,512] -> QT [128, (h,b)]; k_new -> KTn (f16)
            for h in range(HQ):
                tp = pp0.tile([128, B], F32, tag="tp", bufs=2)
                nc.tensor.transpose(
                    tp[:], q_s[:, h * HD:(h + 1) * HD], ident[:]
                )
                nc.vector.tensor_copy(QT[:, h * B:(h + 1) * B], tp[:])
            tpk = pp0.tile([128, B], F32, tag="tp", bufs=2)
            nc.tensor.transpose(tpk[:], kn_s[:], ident[:])
            nc.vector.tensor_copy(KTnh[:], tpk[:])
            nc.vector.tensor_copy(QTh[:], QT[:])

        # ---------------- phase 1: attention over the cache ----------------
        QTh3 = QTh[:].rearrange("p (h b) -> p b h", b=B)   # [128, b, 4]
        vrowh3 = vrowh[:].rearrange("p (b c) -> p b c", c=HD)
        AT3 = AT[:].rearrange("p (h b) -> p b h", b=B)

        with (
            tc.tile_pool(name="ptpool", bufs=2) as ptpool,
            tc.tile_pool(name="small", bufs=2) as small,
            tc.tile_pool(name="stpsum", bufs=2, space="PSUM") as stpsum,
            tc.tile_pool(name="opsum", bufs=2, space="PSUM") as opsum,
            tc.tile_pool(name="denpsum", bufs=1, space="PSUM") as denpsum,
            tc.tile_pool(name="ttpsum", bufs=2, space="PSUM") as ttpsum,
        ):
            for b in range(B):
                ktb = ktiles.pop(b)
                vb = vtiles.pop(b)

                # scores^T tiles: [t'(128), h(4)] per cache tile + new token
                stp = stpsum.tile([128, SW], F32, tag="stp")
                qb = QTh3[:, b, :]
                nc.tensor.matmul(
                    stp[0:1, 4 * NT:SW], KTnh[:, b:b + 1], qb,
                    start=True, stop=True,
                )
                for n in range(NT):
                    nc.tensor.matmul(
                        stp[:, 4 * n:4 * n + 4],
                        ktb[:, 128 * n:128 * (n + 1)],
                        qb,
                        start=True, stop=True,
                    )

                pt = ptpool.tile([128, SW], F16, tag="pt")
                nc.scalar.activation(pt[:, 0:4 * NT], stp[:, 0:4 * NT], Exp, scale=SCALE)
                nc.scalar.activation(
                    pt[0:1, 4 * NT:SW], stp[0:1, 4 * NT:SW], Exp, scale=SCALE,
                )

                # out^T [h(4), 128]: accumulate cache tiles + new token
                op = opsum.tile([HQ, HD], F32, tag="op")
                nc.tensor.matmul(
                    op[:], pt[0:1, 4 * NT:SW], vrowh3[0:1, b, :],
                    start=True, stop=False,
                )
                for n in range(NT):
                    nc.tensor.matmul(
                        op[:],
                        pt[:, 4 * n:4 * n + 4],
                        vb[:, 128 * n:128 * (n + 1)],
                        start=False, stop=(n == NT - 1),
                    )

                # softmax denominators: ones.T @ P -> [1, (g h)], reduce g
                dps = denpsum.tile([1, SW], F32, tag="dps")
                nc.tensor.matmul(
                    dps[0:1, 0:4 * NT], ones_h[:], pt[:, 0:4 * NT],
                    start=True, stop=True,
                )
                nc.tensor.matmul(
                    dps[0:1, 4 * NT:SW], ones_h[0:1, 0:1], pt[0:1, 4 * NT:SW],
                    start=True, stop=True,
                )
                dred = small.tile([1, HQ], F32, tag="dred")
                nc.vector.reduce_sum(
                    dred[:].rearrange("p h -> p h ()"),
                    dps[:].rearrange("p (g h) -> p h g", h=HQ),
                    axis=mybir.AxisListType.X,
                )
                dent = ttpsum.tile([HQ, 1], F32, tag="tt")
                nc.tensor.matmul(dent[:], dred[:], ident[0:1, 0:1],
                                 start=True, stop=True)

                rc = small.tile([HQ, 1], F32, tag="rc")
                nc.vector.reciprocal(rc[:], dent[:])
                ao = small.tile([HQ, HD], F32, tag="ao")
                nc.vector.tensor_scalar(
                    out=ao[:], in0=op[:], scalar1=rc[:], scalar2=None, op0=Mult
                )

                tt = ttpsum.tile([128, HQ], F32, tag="tt")
                nc.tensor.transpose(tt[:], ao[:], ident[0:HQ, 0:HQ])
                nc.vector.tensor_copy(AT3[:, b, :], tt[:])

                if b + KV_DEPTH < B:
                    issue_kv(b + KV_DEPTH)

        # wo f16 chunks queue behind the whole K/V FIFO, contiguous reads
        # (8KB per partition), n-low halves first across all 4 head chunks
        # so phase-2 work for the first output columns starts while the
        # n-high halves are still streaming.
        wo_src = woT[:].rearrange("(c p) n -> c p n", p=128)
        for half in range(2):
            for c in range(4):
                nc.gpsimd.dma_start(
                    wo_h[:, c * DIM + 2048 * half:c * DIM + 2048 * (half + 1)],
                    wo_src[c, :, 2048 * half:2048 * (half + 1)],
                )

        # ---------------- phase 2: output projection, transposed -----------
        # outT[128n, 16b] per n-tile = sum_c wo_h[:, c*DIM+n-range]^T @ AT_c.
        with tc.tile_pool(name="otpsum", bufs=4, space="PSUM") as otpsum:
            for half in range(2):
                for t in range(16):
                    nt = 16 * half + t
                    otp = otpsum.tile([128, B], F32, tag="otp")
                    for c in range(HQ):
                        nc.tensor.matmul(
                            otp[:],
                            wo_h[:, c * DIM + 128 * nt:c * DIM + 128 * (nt + 1)],
                            AT[:, B * c:B * (c + 1)],
                            start=(c == 0), stop=(c == HQ - 1),
                        )
                    nc.vector.tensor_copy(outT_s[:, B * nt:B * (nt + 1)], otp[:])
                nc.sync.dma_start(
                    out[:, B * 16 * half:B * 16 * (half + 1)],
                    outT_s[:, B * 16 * half:B * 16 * (half + 1)],
                )

_NC = None


def _get_nc():
    global _NC
    if _NC is None:
        _NC = _build_nc()
    return _NC


def make_in_maps(inputs):
    x = np.ascontiguousarray(np.asarray(inputs["x"], dtype=np.float32))
    ck = np.asarray(inputs["cache_k"], dtype=np.float32)
    cv = np.asarray(inputs["cache_v"], dtype=np.float32)
    wq = np.asarray(inputs["wq"], dtype=np.float32)
    wk = np.asarray(inputs["wk"], dtype=np.float32)
    wv = np.asarray(inputs["wv"], dtype=np.float32)
    wo = np.asarray(inputs["wo"], dtype=np.float32)

    xT = np.ascontiguousarray(x.reshape(B, DIM).T)
    wqT = np.ascontiguousarray(wq.T)    # [DIM, H*HD]
    wkT = np.ascontiguousarray(wk.T)    # [DIM, HKV*HD]
    wvT = np.ascontiguousarray(wv.T)

    in_maps = []
    for c in range(NCORES):
        hq0 = HQ * HD * c
        # K^T with columns permuted to the t = 64*p + n interleaved order
        # (matches V's natural contiguous-load partition mapping).
        kTc = ck[:, :, c, :].transpose(0, 2, 1)           # [B, 128d, 8192t]
        kTc = np.ascontiguousarray(
            kTc.reshape(B, HD, 128, NT).transpose(0, 1, 3, 2).reshape(B, HD, T)
        )
        in_maps.append({
            "xT": xT,
            "wqT": np.ascontiguousarray(wqT[:, hq0:hq0 + HQ * HD]),
            "wkT": np.ascontiguousarray(wkT[:, HD * c:HD * (c + 1)]),
            "wvT": np.ascontiguousarray(wvT[:, HD * c:HD * (c + 1)]),
            "woT": np.ascontiguousarray(wo[:, hq0:hq0 + HQ * HD].T),
            "kT": kTc,
            "v": np.ascontiguousarray(cv[:, :, c, :]),
        })
    return in_maps


def run(in_maps, trace=False):
    nc = _get_nc()
    return run_bass_kernel_spmd(nc, in_maps, list(range(NCORES)), trace=trace)


def unscramble_out(o):
    """[128, NDT*B] outT tiles -> [B, DIM] partial output."""
    return np.asarray(o).reshape(128, NDT, B).transpose(1, 0, 2).reshape(DIM, B).T


def kernel(**inputs):
    res = run(make_in_maps(inputs)).results
    acc = np.zeros((B, DIM), dtype=np.float64)
    for r in res:
        acc += unscramble_out(r["out"])
    return acc.astype(np.float32).reshape(B, 1, DIM)
